# revision 1
# baseline (speedup 1.0000x reference)
"""GCN graph classification on 8 Trainium2 NeuronCores (Bass/Tile).

Strategy (dst-partitioned message passing):
  - Nodes are permuted and dealt across 8 cores (and 98 blocks of 128 slots
    per core) snake-wise by in-degree, so per-core / per-block edge counts
    are balanced.
  - Layer 0 collapses to an outer product (input features are all-ones):
    x1 = relu(a * W0 + b0) with a = dinv * segsum(dinv[src]) computed on host.
  - Each conv layer: y = dinv * (x @ W) computed per-core on its node slice
    (PE transpose + matmul), written to a bf16 node table slice, AllGathered
    (in 4 chunks) to every core's full table in DRAM.
  - Aggregation z[v] = sum_{e->v} y[src_e] runs as dma_gather (int16 indices,
    4 src windows of 32768 rows, 4 SWDGE queues) + one-hot selection matmuls
    accumulating each 128-dst-slot block in PSUM. Eviction fuses
    x' = relu(dinv * z + b).
  - Mean-pooling per graph via selection matmuls + small AllReduce; the
    classifier head and log_softmax run on-chip.
"""
import sys

sys.path.insert(0, "/opt/trn_rl_repo")

import numpy as np
import ml_dtypes

import concourse.bass as bass
import concourse.bacc as bacc
import concourse.mybir as mybir
import concourse.tile as tile
from concourse.bass_utils import run_bass_kernel_spmd

# problem constants (hardcoded per spec)
N = 100000
E = 1600000
G = 512
H = 128
C = 10
NC = 8
NB = 98                # blocks per core
S = NB * 128           # node slots per core = 12544
NPAD = NC * S          # padded node/table rows = 100352
WIN = 32768            # src window (int16 index range)
NWIN = 4
GRP = 4                # blocks per gather group
NGRP = (NB + GRP - 1) // GRP  # 25 (last group has 2 blocks)
# AllGather chunking: blocks [25,25,25,23]
AG_BLKS = [25, 25, 25, 23]
AG_ROWS = [b * 128 for b in AG_BLKS]          # rows per core per chunk
AG_SLOT0 = [0, 3200, 6400, 9600]              # first slot of chunk
AG_BASE = [0, 25600, 51200, 76800]            # table row base of chunk

F32 = mybir.dt.float32
BF16 = mybir.dt.bfloat16
I16 = mybir.dt.int16
NP_BF16 = ml_dtypes.bfloat16

BF16_TABLE = True
TDT = BF16 if BF16_TABLE else F32
NP_TDT = NP_BF16 if BF16_TABLE else np.float32


def _blocks_of_group(g):
    return range(g * GRP, min((g + 1) * GRP, NB))


def preprocess(edge_index, batch):
    """Host-side graph preprocessing. Returns per-core input arrays and the
    (SPMD-uniform) gather/matmul schedule."""
    edge_index = np.asarray(edge_index, dtype=np.int64)
    batch = np.asarray(batch, dtype=np.int64)

    loop = np.arange(N, dtype=np.int64)
    src = np.concatenate([edge_index[0], loop])
    dst = np.concatenate([edge_index[1], loop])
    EE = src.shape[0]

    deg = np.bincount(dst, minlength=N).astype(np.float64)
    dinv = np.where(deg > 0, 1.0 / np.sqrt(deg), 0.0)
    csum = np.bincount(dst, weights=dinv[src], minlength=N)
    a = (dinv * csum).astype(np.float32)
    dinv32 = dinv.astype(np.float32)

    # node -> (core, slot): snake deal by descending degree
    order = np.argsort(-deg, kind="stable")
    pos = np.arange(N)
    p16 = pos % 16
    core_r = np.where(p16 < 8, p16, 15 - p16)
    j_r = (pos // 16) * 2 + (p16 >= 8)
    core = np.empty(N, dtype=np.int64)
    jwc = np.empty(N, dtype=np.int64)
    core[order] = core_r
    jwc[order] = j_r
    pas = jwc // NB
    r = jwc % NB
    blk = np.where(pas % 2 == 0, r, NB - 1 - r)
    slot = blk * 128 + pas
    assert pas.max() < 128

    # table row
    t = np.minimum(slot // 3200, 3)
    rows_t = np.array(AG_ROWS)[t]
    base_t = np.array(AG_BASE)[t]
    slot0_t = np.array(AG_SLOT0)[t]
    tr = base_t + core * rows_t + (slot - slot0_t)
    assert tr.min() >= 0 and tr.max() < NPAD

    # per-slot arrays [NC, 128, NB]
    dinv_sl = np.zeros((NC, S), dtype=np.float32)
    a_sl = np.zeros((NC, S), dtype=np.float32)
    batc_sl = np.full((NC, S), -1.0, dtype=np.float32)
    dinv_sl[core, slot] = dinv32
    a_sl[core, slot] = a
    batc_sl[core, slot] = batch.astype(np.float32)

    def to_pj(x):  # [NC, S] -> [NC, 128, NB]  ([p, J] with slot = J*128+p)
        return np.ascontiguousarray(x.reshape(NC, NB, 128).transpose(0, 2, 1))

    dinv_pj = to_pj(dinv_sl)
    a_pj = to_pj(a_sl)
    batc_pj = to_pj(batc_sl)

    # edges
    ecore = core[dst]
    eslot = slot[dst]
    eJ = eslot // 128
    eP = (eslot % 128).astype(np.float32)
    etr = tr[src]
    eq = etr // WIN
    eidx = (etr - eq * WIN).astype(np.int16)

    key = (ecore * NB + eJ) * NWIN + eq
    cnt = np.bincount(key, minlength=NC * NB * NWIN).reshape(NC, NB, NWIN)
    cq = np.ceil(cnt.max(axis=0) / 128).astype(np.int64)  # [NB, NWIN]

    # stream layout: for g: for q: for J in group: cq[J,q]*128 tokens
    seg_tok0 = np.zeros((NB, NWIN), dtype=np.int64)
    gathers = []  # (g, q, tok0, ntok)
    tok = 0
    for g in range(NGRP):
        for q in range(NWIN):
            t0 = tok
            for J in _blocks_of_group(g):
                seg_tok0[J, q] = tok
                tok += cq[J, q] * 128
            if tok > t0:
                gathers.append((g, q, t0, tok - t0))
    TOK = tok
    assert TOK % 128 == 0

    # scatter edges into per-core streams
    ordk = np.argsort(key, kind="stable")
    skey = key[ordk]
    first = np.searchsorted(skey, skey)  # index of first occurrence
    rank = np.arange(EE) - first
    p_stream = seg_tok0[eJ[ordk], eq[ordk]] + rank

    gidx = np.zeros((NC, TOK), dtype=np.int16)
    dloc = np.full((NC, TOK), -1.0, dtype=np.float32)
    gidx[ecore[ordk], p_stream] = eidx[ordk]
    dloc[ecore[ordk], p_stream] = eP[ordk]

    # device layouts
    # idx: [128, TOK//16] int16, rows 16k+j replicated; element j of stream at
    #      [j%16, j//16]
    g16 = np.ascontiguousarray(gidx.reshape(NC, TOK // 16, 16).transpose(0, 2, 1))
    gidx_dev = np.tile(g16, (1, 8, 1))  # [NC, 128, TOK//16]
    # dloc: [128, TOK//128], token j at [j%128, j//128]
    dloc_dev = np.ascontiguousarray(
        dloc.reshape(NC, TOK // 128, 128).transpose(0, 2, 1)
    ).astype(NP_TDT)

    cntg = np.bincount(batch, minlength=G).astype(np.float32)
    invcnt = (1.0 / np.maximum(cntg, 1.0)).reshape(4, 128).T.copy()  # [128, 4]

    sched = {
        "cq": cq,
        "seg_tok0": seg_tok0,
        "gathers": gathers,
        "TOK": TOK,
    }
    percore = {
        "gidx": gidx_dev,
        "dloc": dloc_dev,
        "dinv_pj": dinv_pj,
        "a_pj": a_pj,
        "batc_pj": batc_pj,
    }
    return sched, percore, invcnt


def build_program(sched):
    import os
    SKIP_AGG = bool(int(os.environ.get("KGCN_SKIP_AGG", "0")))
    SKIP_AG = bool(int(os.environ.get("KGCN_SKIP_AG", "0")))
    SKIP_A = bool(int(os.environ.get("KGCN_SKIP_A", "0")))
    NLAYER = int(os.environ.get("KGCN_NLAYER", "2"))
    cq = sched["cq"]
    seg_tok0 = sched["seg_tok0"]
    gathers = sched["gathers"]
    TOK = sched["TOK"]

    nc = bacc.Bacc(
        "TRN2",
        target_bir_lowering=False,
        debug=False,
        num_devices=NC,
        num_swdge_queues=4,
    )

    # inputs
    din = {}
    din["gidx"] = nc.dram_tensor("gidx", [128, TOK // 16], I16, kind="ExternalInput")
    din["dloc"] = nc.dram_tensor("dloc", [128, TOK // 128], TDT, kind="ExternalInput")
    din["dinv"] = nc.dram_tensor("dinv", [128, NB], F32, kind="ExternalInput")
    din["acol"] = nc.dram_tensor("acol", [128, NB], F32, kind="ExternalInput")
    din["batchf"] = nc.dram_tensor("batchf", [128, NB], F32, kind="ExternalInput")
    din["W1"] = nc.dram_tensor("W1", [H, H], F32, kind="ExternalInput")
    din["W2"] = nc.dram_tensor("W2", [H, H], F32, kind="ExternalInput")
    din["Wp"] = nc.dram_tensor("Wp", [H, C], F32, kind="ExternalInput")
    din["W0r"] = nc.dram_tensor("W0r", [128, H], F32, kind="ExternalInput")
    din["b0r"] = nc.dram_tensor("b0r", [128, H], F32, kind="ExternalInput")
    din["b1r"] = nc.dram_tensor("b1r", [128, H], F32, kind="ExternalInput")
    din["b2r"] = nc.dram_tensor("b2r", [128, H], F32, kind="ExternalInput")
    din["bpr"] = nc.dram_tensor("bpr", [128, C], F32, kind="ExternalInput")
    din["ident"] = nc.dram_tensor("ident", [128, 128], F32, kind="ExternalInput")
    din["iotar"] = nc.dram_tensor("iotar", [128, 128], TDT, kind="ExternalInput")
    din["giota"] = nc.dram_tensor("giota", [128, G], F32, kind="ExternalInput")
    din["invc"] = nc.dram_tensor("invc", [128, 4], F32, kind="ExternalInput")
    out = nc.dram_tensor("out", [G, C], F32, kind="ExternalOutput")

    # internal DRAM
    y_slice = nc.dram_tensor("y_slice", [S, H], TDT)
    y_full = nc.dram_tensor("y_full", [NPAD, H], TDT, addr_space="Shared")
    pp = nc.dram_tensor("pp", [G, H], F32)
    pooled = nc.dram_tensor("pooled", [G, H], F32, addr_space="Shared")

    rg = [list(range(NC))]

    from contextlib import ExitStack
    ctx = ExitStack()
    with tile.TileContext(nc) as tc, ctx:
        cpool = ctx.enter_context(tc.tile_pool(name="consts", bufs=1))
        msgp = ctx.enter_context(tc.tile_pool(name="msg", bufs=6))
        selp = ctx.enter_context(tc.tile_pool(name="sel", bufs=4))
        wrk = ctx.enter_context(tc.tile_pool(name="wrk", bufs=4))
        # PSUM budget is 8 banks total: xt(2) + h(2) + z(4) = 8. The pooling
        # accumulators reuse tag "z" (4 live tiles), the head reuses xt/h.
        ps_a = ctx.enter_context(tc.tile_pool(name="psA", bufs=2, space="PSUM"))
        ps_b = ctx.enter_context(tc.tile_pool(name="psB", bufs=4, space="PSUM"))

        # resident tiles
        def load_const(name, shape, dt):
            t = cpool.tile(shape, dt, tag=name)
            nc.sync.dma_start(out=t[:], in_=din[name][:])
            return t

        gidx_sb = load_const("gidx", [128, TOK // 16], I16)
        dloc_sb = load_const("dloc", [128, TOK // 128], TDT)
        dinv_sb = load_const("dinv", [128, NB], F32)
        acol_sb = load_const("acol", [128, NB], F32)
        batc_sb = load_const("batchf", [128, NB], F32)
        w_sb = {
            1: load_const("W1", [H, H], F32),
            2: load_const("W2", [H, H], F32),
        }
        wp_sb = load_const("Wp", [H, C], F32)
        w0r_sb = load_const("W0r", [128, H], F32)
        br_sb = {
            0: load_const("b0r", [128, H], F32),
            1: load_const("b1r", [128, H], F32),
            2: load_const("b2r", [128, H], F32),
        }
        bpr_sb = load_const("bpr", [128, C], F32)
        id_sb = load_const("ident", [128, 128], F32)
        iot_sb = load_const("iotar", [128, 128], TDT)
        gio_sb = load_const("giota", [128, G], F32)
        ivc_sb = load_const("invc", [128, 4], F32)

        x_sb = cpool.tile([128, S], F32, tag="x")  # node features, [p, J*128+f]

        def xblk(J):
            return x_sb[:, J * 128:(J + 1) * 128]

        # ---- layer 0: x1 = relu(a * W0 + b0) ----
        for J in range(NB):
            t0 = wrk.tile([128, H], F32, tag="l0")
            nc.vector.scalar_tensor_tensor(
                out=t0[:],
                in0=w0r_sb[:],
                scalar=acol_sb[:, J:J + 1],
                in1=br_sb[0][:],
                op0=mybir.AluOpType.mult,
                op1=mybir.AluOpType.add,
            )
            nc.scalar.activation(xblk(J), t0[:], mybir.ActivationFunctionType.Relu)

        # ---- conv layers ----
        for layer in range(1, NLAYER + 1):
            # phase A: y = dinv * (x @ W) -> y_slice, AllGather in 4 chunks
            for ci in range(4) if not SKIP_A else []:
                J0 = AG_SLOT0[ci] // 128
                for J in range(J0, J0 + AG_BLKS[ci]):
                    xt_ps = ps_a.tile([128, 128], F32, tag="xt")
                    nc.tensor.transpose(out=xt_ps[:], in_=xblk(J), identity=id_sb[:])
                    xt_sb = wrk.tile([128, 128], F32, tag="xt_sb")
                    nc.scalar.copy(xt_sb[:], xt_ps[:])
                    h_ps = ps_a.tile([128, H], F32, tag="h")
                    nc.tensor.matmul(
                        out=h_ps[:], lhsT=xt_sb[:], rhs=w_sb[layer][:],
                        start=True, stop=True,
                    )
                    y_sb = wrk.tile([128, H], TDT, tag="y")
                    nc.scalar.mul(y_sb[:], h_ps[:], mul=dinv_sb[:, J:J + 1])
                    nc.sync.dma_start(
                        out=y_slice[J * 128:(J + 1) * 128, :], in_=y_sb[:]
                    )
                r0 = AG_SLOT0[ci]
                nrow = AG_ROWS[ci]
                if not SKIP_AG:
                    nc.gpsimd.collective_compute(
                        "AllGather",
                        mybir.AluOpType.bypass,
                        replica_groups=rg,
                        ins=[y_slice[r0:r0 + nrow, :]],
                        outs=[y_full[AG_BASE[ci]:AG_BASE[ci] + NC * nrow, :]],
                    )

            # phase B/C: aggregate + relu(dinv*z + b)
            qctr = 0
            for g in range(NGRP) if not SKIP_AGG else []:
                blocks = list(_blocks_of_group(g))
                # issue the group's gathers (one per window with tokens)
                msg_t = {}
                sel_t = {}
                gtok0 = {}
                for (gg, q, tok0, ntok) in gathers:
                    if gg != g:
                        continue
                    nslots = ntok // 128
                    mt = msgp.tile([128, nslots * H], TDT, tag="msg")
                    wq = y_full[q * WIN:min((q + 1) * WIN, NPAD), :]
                    nc.gpsimd.dma_gather(
                        out_ap=mt[:].rearrange("p (s e) -> p s e", e=H),
                        in_ap=wq,
                        idxs_ap=gidx_sb[:, tok0 // 16:(tok0 + ntok) // 16],
                        num_idxs=ntok,
                        num_idxs_reg=ntok,
                        elem_size=H,
                        queue_num=qctr % 4,
                        single_packet=False,
                    )
                    qctr += 1
                    st = selp.tile([128, nslots * 128], TDT, tag="sel")
                    nc.vector.tensor_tensor(
                        out=st[:].rearrange("p (s e) -> p s e", e=128),
                        in0=dloc_sb[:, tok0 // 128:(tok0 + ntok) // 128, None]
                        .to_broadcast([128, nslots, 128]),
                        in1=iot_sb[:, None, :].to_broadcast([128, nslots, 128]),
                        op=mybir.AluOpType.is_equal,
                    )
                    msg_t[q] = mt
                    sel_t[q] = st
                    gtok0[q] = tok0

                for J in blocks:
                    zp = ps_b.tile([128, H], F32, tag="z")
                    tot = int(cq[J].sum())
                    done = 0
                    for q in range(NWIN):
                        nch = int(cq[J, q])
                        if nch == 0:
                            continue
                        s0 = (seg_tok0[J, q] - gtok0[q]) // 128
                        for i in range(nch):
                            s = s0 + i
                            nc.tensor.matmul(
                                out=zp[:],
                                lhsT=sel_t[q][:, (s * 128):(s + 1) * 128],
                                rhs=msg_t[q][:].rearrange(
                                    "p (s e) -> p s e", e=H)[:, s, :],
                                start=(done == 0),
                                stop=(done == tot - 1),
                            )
                            done += 1
                    t1 = wrk.tile([128, H], F32, tag="pc")
                    nc.vector.scalar_tensor_tensor(
                        out=t1[:],
                        in0=zp[:],
                        scalar=dinv_sb[:, J:J + 1],
                        in1=br_sb[layer][:],
                        op0=mybir.AluOpType.mult,
                        op1=mybir.AluOpType.add,
                    )
                    nc.scalar.activation(
                        xblk(J), t1[:], mybir.ActivationFunctionType.Relu
                    )

        # ---- pooling ----
        psum_pool = [
            ps_b.tile([128, H], F32, tag="z", name=f"poolacc{gb}")
            for gb in range(4)
        ]
        for J in range(NB):
            selg = wrk.tile([128, G], F32, tag="selg")
            nc.vector.tensor_tensor(
                out=selg[:],
                in0=batc_sb[:, J:J + 1].to_broadcast([128, G]),
                in1=gio_sb[:],
                op=mybir.AluOpType.is_equal,
            )
            for gb in range(4):
                nc.tensor.matmul(
                    out=psum_pool[gb][:],
                    lhsT=selg[:, gb * 128:(gb + 1) * 128],
                    rhs=xblk(J),
                    start=(J == 0),
                    stop=(J == NB - 1),
                )
        for gb in range(4):
            t2 = wrk.tile([128, H], F32, tag="ppev")
            nc.scalar.copy(t2[:], psum_pool[gb][:])
            nc.sync.dma_start(out=pp[gb * 128:(gb + 1) * 128, :], in_=t2[:])
        nc.gpsimd.collective_compute(
            "AllReduce",
            mybir.AluOpType.add,
            replica_groups=rg,
            ins=[pp[:]],
            outs=[pooled[:]],
        )

        # ---- head + log_softmax ----
        for gb in range(4):
            pl = wrk.tile([128, H], F32, tag="pl")
            nc.sync.dma_start(out=pl[:], in_=pooled[gb * 128:(gb + 1) * 128, :])
            plm = wrk.tile([128, H], F32, tag="plm")
            nc.scalar.mul(plm[:], pl[:], mul=ivc_sb[:, gb:gb + 1])
            pt_ps = ps_a.tile([128, 128], F32, tag="xt")
            nc.tensor.transpose(out=pt_ps[:], in_=plm[:], identity=id_sb[:])
            pt_sb = wrk.tile([128, 128], F32, tag="pts")
            nc.scalar.copy(pt_sb[:], pt_ps[:])
            lg_ps = ps_a.tile([128, C], F32, tag="h")
            nc.tensor.matmul(
                out=lg_ps[:], lhsT=pt_sb[:], rhs=wp_sb[:], start=True, stop=True
            )
            tl = wrk.tile([128, C], F32, tag="tl")
            nc.vector.tensor_tensor(
                out=tl[:], in0=lg_ps[:], in1=bpr_sb[:], op=mybir.AluOpType.add
            )
            mx = wrk.tile([128, 1], F32, tag="mx")
            nc.vector.tensor_reduce(
                out=mx[:], in_=tl[:], axis=mybir.AxisListType.X,
                op=mybir.AluOpType.max,
            )
            nmx = wrk.tile([128, 1], F32, tag="nmx")
            nc.vector.tensor_scalar_mul(nmx[:], mx[:], -1.0)
            ex = wrk.tile([128, C], F32, tag="ex")
            ssum = wrk.tile([128, 1], F32, tag="ssum")
            nc.scalar.activation(
                ex[:], tl[:], mybir.ActivationFunctionType.Exp,
                bias=nmx[:, :1], accum_out=ssum[:],
            )
            lns = wrk.tile([128, 1], F32, tag="lns")
            nc.scalar.activation(lns[:], ssum[:], mybir.ActivationFunctionType.Ln)
            ofs = wrk.tile([128, 1], F32, tag="ofs")
            nc.vector.tensor_tensor(
                out=ofs[:], in0=nmx[:], in1=lns[:], op=mybir.AluOpType.subtract
            )
            fin = wrk.tile([128, C], F32, tag="fin")
            nc.vector.tensor_scalar_add(fin[:], tl[:], ofs[:, :1])
            nc.sync.dma_start(out=out[gb * 128:(gb + 1) * 128, :], in_=fin[:])

    nc.compile()
    return nc


_CACHE = {}


def kernel(edge_index, batch, W0, b0, W1, b1, W2, b2, Wp, bp):
    edge_index = np.asarray(edge_index, dtype=np.int32)
    batch = np.asarray(batch, dtype=np.int32)
    W0 = np.asarray(W0, dtype=np.float32)
    b0 = np.asarray(b0, dtype=np.float32)
    W1 = np.asarray(W1, dtype=np.float32)
    b1 = np.asarray(b1, dtype=np.float32)
    W2 = np.asarray(W2, dtype=np.float32)
    b2 = np.asarray(b2, dtype=np.float32)
    Wp = np.asarray(Wp, dtype=np.float32)
    bp = np.asarray(bp, dtype=np.float32)

    key = hash((edge_index.tobytes(), batch.tobytes()))
    if key not in _CACHE:
        sched, percore, invcnt = preprocess(edge_index, batch)
        nc = build_program(sched)
        _CACHE[key] = (sched, percore, invcnt, nc)
    sched, percore, invcnt, nc = _CACHE[key]

    consts = {
        "W1": W1,
        "W2": W2,
        "Wp": Wp,
        "W0r": np.tile(W0.reshape(1, H), (128, 1)),
        "b0r": np.tile(b0.reshape(1, H), (128, 1)),
        "b1r": np.tile(b1.reshape(1, H), (128, 1)),
        "b2r": np.tile(b2.reshape(1, H), (128, 1)),
        "bpr": np.tile(bp.reshape(1, C), (128, 1)),
        "ident": np.eye(128, dtype=np.float32),
        "iotar": np.tile(np.arange(128, dtype=np.float32).astype(NP_TDT).reshape(1, 128), (128, 1)),
        "giota": np.tile(np.arange(G, dtype=np.float32).reshape(1, G), (128, 1)),
        "invc": invcnt,
    }
    consts = {k: np.ascontiguousarray(v) for k, v in consts.items()}

    in_maps = []
    for c in range(NC):
        m = {
            "gidx": percore["gidx"][c],
            "dloc": percore["dloc"][c],
            "dinv": percore["dinv_pj"][c],
            "acol": percore["a_pj"][c],
            "batchf": percore["batc_pj"][c],
        }
        m.update(consts)
        in_maps.append(m)

    import os
    trace = bool(int(os.environ.get("KGCN_TRACE", "0")))
    res = run_bass_kernel_spmd(
        nc, in_maps, core_ids=list(range(NC)), trace=trace
    )
    kernel.last_results = res
    return res.results[0]["out"]



# revision 3
# speedup vs baseline: 1.2481x; 1.2481x over previous
"""GCN graph classification on 8 Trainium2 NeuronCores (Bass/Tile).

Strategy (dst-partitioned message passing, Pool-saturated pipeline):
  - Nodes are dealt across 8 cores x 98 blocks of 128 slots, degree-banded so
    per-core / per-block edge counts are balanced.
  - Layer 0 collapses to an outer product (input features are all-ones):
    x1 = relu(a * W0 + b0) with a = dinv * segsum(dinv[src]) computed on host.
  - Node table y = dinv * (x @ W) (bf16) lives in DRAM, AllGathered in 4
    chunks whose row ranges exactly match the 4 int16 gather windows
    ([4096,4096,4096,256] rows per core), so gather window q depends only on
    AllGather chunk q.
  - Aggregation z[v] = sum_{e->v} y[src_e] runs window-major (w0,w1,w3,w2):
    dma_gather (int16 idx, 4 SWDGE queues) + one-hot selection matmuls per
    (dst-block, window) segment accumulated in PSUM, then added into an SBUF
    accumulator. Self-loop edges are excluded from the streams and folded in
    algebraically (k_v * y[v]) with the first segment add.
  - The last window (w2) is consumed in block order, so per-block eviction
    x' = relu(dinv*z + b), the next layer's y computation, AllGather chunks,
    and the pooling matmuls all cascade underneath the gather stream - the
    Pool engine (the descriptor-generation bottleneck) never idles.
  - Mean-pooling per graph via selection matmuls + small AllReduce; the
    classifier head and log_softmax run on-chip.
"""
import sys

sys.path.insert(0, "/opt/trn_rl_repo")

import numpy as np
import ml_dtypes

import concourse.bass as bass
import concourse.bacc as bacc
import concourse.mybir as mybir
import concourse.tile as tile
from concourse.bass_utils import run_bass_kernel_spmd

# problem constants (hardcoded per spec)
N = 100000
E = 1600000
G = 512
H = 128
C = 10
NC = 8
NB = 98                # blocks per core
S = NB * 128           # node slots per core = 12544
NPAD = NC * S          # padded node/table rows = 100352
WIN = 32768            # src window (int16 index range)
NWIN = 4
NS = 24                # gather size in 128-token chunks
Q_ORDER = [0, 1, 3, 2]   # window issue order (w2 last -> evictions cascade)
# AllGather chunking == gather windows: rows per core per chunk
AG_ROWS = [4096, 4096, 4096, 256]
AG_SLOT0 = [0, 4096, 8192, 12288]
AG_BASE = [0, 32768, 65536, 98304]

F32 = mybir.dt.float32
BF16 = mybir.dt.bfloat16
I16 = mybir.dt.int16
NP_BF16 = ml_dtypes.bfloat16


def preprocess(edge_index, batch):
    """Host-side graph preprocessing. Returns per-core input arrays and the
    (SPMD-uniform) gather/matmul schedule."""
    edge_index = np.asarray(edge_index, dtype=np.int64)
    batch = np.asarray(batch, dtype=np.int64)

    loop = np.arange(N, dtype=np.int64)
    src_all = np.concatenate([edge_index[0], loop])
    dst_all = np.concatenate([edge_index[1], loop])

    deg = np.bincount(dst_all, minlength=N).astype(np.float64)
    dinv = np.where(deg > 0, 1.0 / np.sqrt(deg), 0.0)
    csum = np.bincount(dst_all, weights=dinv[src_all], minlength=N)
    a = (dinv * csum).astype(np.float32)
    dinv32 = dinv.astype(np.float32)

    # self-edges (incl. the added loops) handled algebraically on-device
    sm = src_all == dst_all
    selfw = np.bincount(dst_all[sm], minlength=N).astype(np.float32)
    src = src_all[~sm]
    dst = dst_all[~sm]
    EE = src.shape[0]

    # node -> (core, slot): snake deal by descending degree
    order = np.argsort(-deg, kind="stable")
    pos = np.arange(N)
    p16 = pos % 16
    core_r = np.where(p16 < 8, p16, 15 - p16)
    j_r = (pos // 16) * 2 + (p16 >= 8)
    core = np.empty(N, dtype=np.int64)
    jwc = np.empty(N, dtype=np.int64)
    core[order] = core_r
    jwc[order] = j_r
    pas = jwc // NB
    r = jwc % NB
    blk = np.where(pas % 2 == 0, r, NB - 1 - r)
    slot = blk * 128 + pas
    assert pas.max() < 128

    # table row: chunk t holds slots [AG_SLOT0[t], +AG_ROWS[t]) of every core
    t = np.minimum(slot // 4096, 3)
    rows_t = np.array(AG_ROWS)[t]
    base_t = np.array(AG_BASE)[t]
    slot0_t = np.array(AG_SLOT0)[t]
    tr = base_t + core * rows_t + (slot - slot0_t)
    assert tr.min() >= 0 and tr.max() < NPAD

    # per-slot arrays [NC, 128, NB]
    def scatter_sl(vals, fill=0.0):
        out = np.full((NC, S), fill, dtype=np.float32)
        out[core, slot] = vals
        return out

    def to_pj(x):  # [NC, S] -> [NC, 128, NB]  ([p, J] with slot = J*128+p)
        return np.ascontiguousarray(x.reshape(NC, NB, 128).transpose(0, 2, 1))

    dinv_pj = to_pj(scatter_sl(dinv32))
    a_pj = to_pj(scatter_sl(a))
    selfw_pj = to_pj(scatter_sl(selfw))
    batc_pj = to_pj(scatter_sl(batch.astype(np.float32), fill=-1.0))

    # edges -> (core, block, window)
    ecore = core[dst]
    eslot = slot[dst]
    eJ = eslot // 128
    eP = (eslot % 128).astype(np.float32)
    etr = tr[src]
    eq = np.where(etr >= AG_BASE[3], 3, etr // WIN)
    eidx = (etr - np.array(AG_BASE)[eq]).astype(np.int16)
    assert eidx.min() >= 0

    key = (ecore * NB + eJ) * NWIN + eq
    cnt = np.bincount(key, minlength=NC * NB * NWIN).reshape(NC, NB, NWIN)
    cq = np.ceil(cnt.max(axis=0) / 128).astype(np.int64)  # [NB, NWIN] chunks

    # stream layout: for q in Q_ORDER: for J: cq[J,q]*128 tokens
    seg_tok0 = np.zeros((NB, NWIN), dtype=np.int64)
    win_chunk0 = {}        # q -> first chunk index of window stream
    chunk_info = []        # per global chunk: (q, J, ci, nci)
    tok = 0
    for q in Q_ORDER:
        win_chunk0[q] = tok // 128
        for J in range(NB):
            seg_tok0[J, q] = tok
            for ci in range(int(cq[J, q])):
                chunk_info.append((q, J, ci, int(cq[J, q])))
            tok += int(cq[J, q]) * 128
    TOK = tok
    assert TOK % 128 == 0

    # gathers: per window, NS-chunk pieces
    gathers = []  # (q, chunk0, nchunks)
    for q in Q_ORDER:
        c0 = win_chunk0[q]
        nch = int(cq[:, q].sum())
        for s in range(c0, c0 + nch, NS):
            gathers.append((q, s, min(NS, c0 + nch - s)))

    # scatter edges into per-core streams
    ordk = np.argsort(key, kind="stable")
    skey = key[ordk]
    first = np.searchsorted(skey, skey)
    rank = np.arange(EE) - first
    p_stream = seg_tok0[eJ[ordk], eq[ordk]] + rank

    gidx = np.zeros((NC, TOK), dtype=np.int16)
    dloc = np.full((NC, TOK), -1.0, dtype=np.float32)
    gidx[ecore[ordk], p_stream] = eidx[ordk]
    dloc[ecore[ordk], p_stream] = eP[ordk]

    # device layouts
    g16 = np.ascontiguousarray(gidx.reshape(NC, TOK // 16, 16).transpose(0, 2, 1))
    gidx_dev = np.tile(g16, (1, 8, 1))  # [NC, 128, TOK//16]
    dloc_dev = np.ascontiguousarray(
        dloc.reshape(NC, TOK // 128, 128).transpose(0, 2, 1)
    ).astype(NP_BF16)

    cntg = np.bincount(batch, minlength=G).astype(np.float32)
    invcnt = (1.0 / np.maximum(cntg, 1.0)).reshape(4, 128).T.copy()  # [128, 4]

    # per-J first/last window in issue order (for self-add / eviction)
    first_q = np.full(NB, -1, dtype=np.int64)
    last_q = np.full(NB, -1, dtype=np.int64)
    for J in range(NB):
        qs = [q for q in Q_ORDER if cq[J, q] > 0]
        assert qs, f"block {J} has no edges"
        first_q[J] = qs[0]
        last_q[J] = qs[-1]

    sched = {
        "cq": cq,
        "gathers": gathers,
        "chunk_info": chunk_info,
        "TOK": TOK,
        "first_q": first_q,
        "last_q": last_q,
    }
    percore = {
        "gidx": gidx_dev,
        "dloc": dloc_dev,
        "dinv_pj": dinv_pj,
        "a_pj": a_pj,
        "selfw_pj": selfw_pj,
        "batc_pj": batc_pj,
    }
    return sched, percore, invcnt


def build_program(sched):
    gathers = sched["gathers"]
    chunk_info = sched["chunk_info"]
    TOK = sched["TOK"]
    first_q = sched["first_q"]
    last_q = sched["last_q"]

    nc = bacc.Bacc(
        "TRN2",
        target_bir_lowering=False,
        debug=False,
        num_devices=NC,
        num_swdge_queues=4,
    )

    # inputs
    din = {}
    din["gidx"] = nc.dram_tensor("gidx", [128, TOK // 16], I16, kind="ExternalInput")
    din["dloc"] = nc.dram_tensor("dloc", [128, TOK // 128], BF16, kind="ExternalInput")
    din["dinv"] = nc.dram_tensor("dinv", [128, NB], F32, kind="ExternalInput")
    din["acol"] = nc.dram_tensor("acol", [128, NB], F32, kind="ExternalInput")
    din["selfw"] = nc.dram_tensor("selfw", [128, NB], F32, kind="ExternalInput")
    din["batchf"] = nc.dram_tensor("batchf", [128, NB], F32, kind="ExternalInput")
    din["W1"] = nc.dram_tensor("W1", [H, H], BF16, kind="ExternalInput")
    din["W2"] = nc.dram_tensor("W2", [H, H], BF16, kind="ExternalInput")
    din["Wp"] = nc.dram_tensor("Wp", [H, C], F32, kind="ExternalInput")
    din["W0r"] = nc.dram_tensor("W0r", [128, H], F32, kind="ExternalInput")
    din["b0r"] = nc.dram_tensor("b0r", [128, H], F32, kind="ExternalInput")
    din["b1r"] = nc.dram_tensor("b1r", [128, H], F32, kind="ExternalInput")
    din["b2r"] = nc.dram_tensor("b2r", [128, H], F32, kind="ExternalInput")
    din["bpr"] = nc.dram_tensor("bpr", [128, C], F32, kind="ExternalInput")
    din["ident"] = nc.dram_tensor("ident", [128, 128], F32, kind="ExternalInput")
    din["identb"] = nc.dram_tensor("identb", [128, 128], BF16, kind="ExternalInput")
    din["iotar"] = nc.dram_tensor("iotar", [128, 128], BF16, kind="ExternalInput")
    din["giota"] = nc.dram_tensor("giota", [128, G], F32, kind="ExternalInput")
    din["invc"] = nc.dram_tensor("invc", [128, 4], F32, kind="ExternalInput")
    out = nc.dram_tensor("out", [G, C], F32, kind="ExternalOutput")

    # internal DRAM
    y_slice = nc.dram_tensor("y_slice", [S, H], BF16)
    y_full = nc.dram_tensor("y_full", [NPAD, H], BF16, addr_space="Shared")
    pp = nc.dram_tensor("pp", [G, H], F32)
    pooled = nc.dram_tensor("pooled", [G, H], F32, addr_space="Shared")

    rg = [list(range(NC))]

    from contextlib import ExitStack
    ctx = ExitStack()
    with tile.TileContext(nc) as tc, ctx:
        cpool = ctx.enter_context(tc.tile_pool(name="consts", bufs=1))
        msgp = ctx.enter_context(tc.tile_pool(name="msg", bufs=6))
        selp = ctx.enter_context(tc.tile_pool(name="sel", bufs=5))
        wrk = ctx.enter_context(tc.tile_pool(name="wrk", bufs=4))
        # PSUM: 8 banks total = seg(4) + ab(4).  "seg" holds per-(J,window)
        # aggregation partials; "ab" is shared by phase-A transpose/matmul
        # tiles (layer boundaries), the pooling accumulators (layer 2), and
        # the head.
        ps = ctx.enter_context(tc.tile_pool(name="ps", bufs=4, space="PSUM"))

        def load_const(name, shape, dt):
            t = cpool.tile(shape, dt, tag=name, name=name + "_sb")
            nc.sync.dma_start(out=t[:], in_=din[name][:])
            return t

        gidx_sb = load_const("gidx", [128, TOK // 16], I16)
        dloc_sb = load_const("dloc", [128, TOK // 128], BF16)
        dinv_sb = load_const("dinv", [128, NB], F32)
        acol_sb = load_const("acol", [128, NB], F32)
        selfw_sb = load_const("selfw", [128, NB], F32)
        batc_sb = load_const("batchf", [128, NB], F32)
        w_sb = {
            1: load_const("W1", [H, H], BF16),
            2: load_const("W2", [H, H], BF16),
        }
        wp_sb = load_const("Wp", [H, C], F32)
        w0r_sb = load_const("W0r", [128, H], F32)
        br_sb = {
            0: load_const("b0r", [128, H], F32),
            1: load_const("b1r", [128, H], F32),
            2: load_const("b2r", [128, H], F32),
        }
        bpr_sb = load_const("bpr", [128, C], F32)
        id_sb = load_const("ident", [128, 128], F32)
        idb_sb = load_const("identb", [128, 128], BF16)
        iot_sb = load_const("iotar", [128, 128], BF16)
        gio_sb = load_const("giota", [128, G], F32)
        ivc_sb = load_const("invc", [128, 4], F32)

        # persistent node state: x / z accumulator (shared buffer) and y
        xz_sb = cpool.tile([128, S], BF16, tag="xz")
        y_sb = cpool.tile([128, S], BF16, tag="y")
        pooled_sb = cpool.tile([128, 4 * H], F32, tag="pooled")

        def xblk(J):
            return xz_sb[:, J * 128:(J + 1) * 128]

        def yblk(J):
            return y_sb[:, J * 128:(J + 1) * 128]

        # ---- emission helpers ------------------------------------------
        def phase_a(J, layer):
            """y[J] = dinv * (x[J] @ W_layer), written to SBUF + y_slice."""
            xt_ps = ps.tile([128, 128], BF16, tag="ab", name="xt_ps")
            nc.tensor.transpose(out=xt_ps[:], in_=xblk(J), identity=idb_sb[:])
            xt_sb = wrk.tile([128, 128], BF16, tag="xt_sb")
            nc.scalar.copy(xt_sb[:], xt_ps[:])
            h_ps = ps.tile([128, H], F32, tag="ab", name="h_ps")
            nc.tensor.matmul(
                out=h_ps[:], lhsT=xt_sb[:], rhs=w_sb[layer][:],
                start=True, stop=True,
            )
            nc.scalar.mul(yblk(J), h_ps[:], mul=dinv_sb[:, J:J + 1])
            nc.sync.dma_start(
                out=y_slice[J * 128:(J + 1) * 128, :], in_=yblk(J)
            )

        def allgather_chunk(t):
            r0 = AG_SLOT0[t]
            nrow = AG_ROWS[t]
            nc.gpsimd.collective_compute(
                "AllGather",
                mybir.AluOpType.bypass,
                replica_groups=rg,
                ins=[y_slice[r0:r0 + nrow, :]],
                outs=[y_full[AG_BASE[t]:AG_BASE[t] + NC * nrow, :]],
            )

        def evict(J, layer):
            """x[J] = relu(dinv * z[J] + b_layer)."""
            t1 = wrk.tile([128, H], F32, tag="pc")
            nc.vector.scalar_tensor_tensor(
                out=t1[:],
                in0=xblk(J),
                scalar=dinv_sb[:, J:J + 1],
                in1=br_sb[layer][:],
                op0=mybir.AluOpType.mult,
                op1=mybir.AluOpType.add,
            )
            nc.scalar.activation(
                xblk(J), t1[:], mybir.ActivationFunctionType.Relu
            )

        # pooling state
        pool_ps = {}      # gb -> live psum tile
        pool_cnt = [0]    # blocks accumulated in current psum octet
        pool_done = [0]   # total blocks pooled

        def pooling(J):
            selg = wrk.tile([128, G], BF16, tag="selg")
            nc.vector.tensor_tensor(
                out=selg[:],
                in0=batc_sb[:, J:J + 1].to_broadcast([128, G]),
                in1=gio_sb[:],
                op=mybir.AluOpType.is_equal,
            )
            if pool_cnt[0] == 0:
                for gb in range(4):
                    pool_ps[gb] = ps.tile(
                        [128, H], F32, tag="ab", name=f"poolps{gb}"
                    )
            octet = min(8, NB - (pool_done[0] - pool_cnt[0]))
            for gb in range(4):
                nc.tensor.matmul(
                    out=pool_ps[gb][:],
                    lhsT=selg[:, gb * 128:(gb + 1) * 128],
                    rhs=xblk(J),
                    start=(pool_cnt[0] == 0),
                    stop=(pool_cnt[0] == octet - 1),
                )
            pool_cnt[0] += 1
            pool_done[0] += 1
            if pool_cnt[0] == octet:
                firstoct = pool_done[0] <= 8
                for gb in range(4):
                    dstp = pooled_sb[:, gb * H:(gb + 1) * H]
                    if firstoct:
                        nc.scalar.copy(dstp, pool_ps[gb][:])
                    else:
                        nc.vector.tensor_tensor(
                            out=dstp, in0=dstp, in1=pool_ps[gb][:],
                            op=mybir.AluOpType.add,
                        )
                pool_cnt[0] = 0

        # ---- layer 0: x1 = relu(a * W0 + b0); phase A for layer 1 ------
        ag_emitted = set()
        for J in range(NB):
            t0 = wrk.tile([128, H], F32, tag="l0")
            nc.vector.scalar_tensor_tensor(
                out=t0[:],
                in0=w0r_sb[:],
                scalar=acol_sb[:, J:J + 1],
                in1=br_sb[0][:],
                op0=mybir.AluOpType.mult,
                op1=mybir.AluOpType.add,
            )
            nc.scalar.activation(xblk(J), t0[:], mybir.ActivationFunctionType.Relu)
            phase_a(J, 1)
            t = min(J // 32, 3)
            if J == min(32 * t + 31, NB - 1):
                allgather_chunk(t)
                if J == NB - 1:
                    allgather_chunk(3)

        # ---- conv layers (aggregation pipeline) ------------------------
        for layer in (1, 2):
            zp_open = {}          # J -> live psum tile for current segment
            ag_pending = []       # (emit_after_gather_idx, chunk_t)
            qctr = 0
            for gi, (q, chunk0, nch) in enumerate(gathers):
                # flush AllGather emissions scheduled for this point
                while ag_pending and ag_pending[0][0] <= gi:
                    allgather_chunk(ag_pending.pop(0)[1])

                mt = msgp.tile([128, NS * H], BF16, tag="msg")
                wq = y_full[AG_BASE[q]:AG_BASE[q] + NC * AG_ROWS[q], :]
                ntok = nch * 128
                nc.gpsimd.dma_gather(
                    out_ap=mt[:, :nch * H].rearrange("p (s e) -> p s e", e=H),
                    in_ap=wq,
                    idxs_ap=gidx_sb[:, chunk0 * 8:(chunk0 + nch) * 8],
                    num_idxs=ntok,
                    num_idxs_reg=ntok,
                    elem_size=H,
                    queue_num=qctr % 4,
                    single_packet=False,
                )
                qctr += 1
                st = selp.tile([128, NS * 128], BF16, tag="sel")
                nc.vector.tensor_tensor(
                    out=st[:, :nch * 128].rearrange("p (s e) -> p s e", e=128),
                    in0=dloc_sb[:, chunk0:chunk0 + nch, None]
                    .to_broadcast([128, nch, 128]),
                    in1=iot_sb[:, None, :].to_broadcast([128, nch, 128]),
                    op=mybir.AluOpType.is_equal,
                )
                for c in range(nch):
                    cq_, J, ci, nci = chunk_info[chunk0 + c]
                    assert cq_ == q
                    if ci == 0:
                        zp_open[J] = ps.tile(
                            [128, H], F32, tag="seg", name=f"seg{layer}_{J}_{q}"
                        )
                    zp = zp_open[J]
                    nc.tensor.matmul(
                        out=zp[:],
                        lhsT=st[:, c * 128:(c + 1) * 128],
                        rhs=mt[:, c * H:(c + 1) * H],
                        start=(ci == 0),
                        stop=(ci == nci - 1),
                    )
                    if ci != nci - 1:
                        continue
                    # segment complete: fold into SBUF accumulator
                    if q == first_q[J]:
                        # z = selfw * y_local + seg   (self-loops folded in)
                        nc.vector.scalar_tensor_tensor(
                            out=xblk(J),
                            in0=yblk(J),
                            scalar=selfw_sb[:, J:J + 1],
                            in1=zp[:],
                            op0=mybir.AluOpType.mult,
                            op1=mybir.AluOpType.add,
                        )
                    else:
                        nc.vector.tensor_tensor(
                            out=xblk(J), in0=xblk(J), in1=zp[:],
                            op=mybir.AluOpType.add,
                        )
                    del zp_open[J]
                    if q != last_q[J]:
                        continue
                    # all windows in: evict and cascade the next stage
                    evict(J, layer)
                    if layer == 1:
                        phase_a(J, 2)
                        t = min(J // 32, 3)
                        if J == min(32 * t + 31, NB - 1):
                            # defer the collective dispatch two gathers to
                            # keep it off the Pool queue's critical path
                            ag_pending.append((gi + 2, t))
                            if J == NB - 1:
                                ag_pending.append((gi + 2, 3))
                    else:
                        pooling(J)
            while ag_pending:
                allgather_chunk(ag_pending.pop(0)[1])
            assert not zp_open

        # ---- pooled -> AllReduce -> head -------------------------------
        for gb in range(4):
            t2 = wrk.tile([128, H], F32, tag="ppev")
            nc.scalar.copy(t2[:], pooled_sb[:, gb * H:(gb + 1) * H])
            nc.sync.dma_start(out=pp[gb * 128:(gb + 1) * 128, :], in_=t2[:])
        nc.gpsimd.collective_compute(
            "AllReduce",
            mybir.AluOpType.add,
            replica_groups=rg,
            ins=[pp[:]],
            outs=[pooled[:]],
        )

        for gb in range(4):
            pl = wrk.tile([128, H], F32, tag="pl")
            nc.sync.dma_start(out=pl[:], in_=pooled[gb * 128:(gb + 1) * 128, :])
            plm = wrk.tile([128, H], F32, tag="plm")
            nc.scalar.mul(plm[:], pl[:], mul=ivc_sb[:, gb:gb + 1])
            pt_ps = ps.tile([128, 128], F32, tag="ab", name="pt_ps")
            nc.tensor.transpose(out=pt_ps[:], in_=plm[:], identity=id_sb[:])
            pt_sb = wrk.tile([128, 128], F32, tag="pts")
            nc.scalar.copy(pt_sb[:], pt_ps[:])
            lg_ps = ps.tile([128, C], F32, tag="ab", name="lg_ps")
            nc.tensor.matmul(
                out=lg_ps[:], lhsT=pt_sb[:], rhs=wp_sb[:], start=True, stop=True
            )
            tl = wrk.tile([128, C], F32, tag="tl")
            nc.vector.tensor_tensor(
                out=tl[:], in0=lg_ps[:], in1=bpr_sb[:], op=mybir.AluOpType.add
            )
            mx = wrk.tile([128, 1], F32, tag="mx")
            nc.vector.tensor_reduce(
                out=mx[:], in_=tl[:], axis=mybir.AxisListType.X,
                op=mybir.AluOpType.max,
            )
            nmx = wrk.tile([128, 1], F32, tag="nmx")
            nc.vector.tensor_scalar_mul(nmx[:], mx[:], -1.0)
            ex = wrk.tile([128, C], F32, tag="ex")
            ssum = wrk.tile([128, 1], F32, tag="ssum")
            nc.scalar.activation(
                ex[:], tl[:], mybir.ActivationFunctionType.Exp,
                bias=nmx[:, :1], accum_out=ssum[:],
            )
            lns = wrk.tile([128, 1], F32, tag="lns")
            nc.scalar.activation(lns[:], ssum[:], mybir.ActivationFunctionType.Ln)
            ofs = wrk.tile([128, 1], F32, tag="ofs")
            nc.vector.tensor_tensor(
                out=ofs[:], in0=nmx[:], in1=lns[:], op=mybir.AluOpType.subtract
            )
            fin = wrk.tile([128, C], F32, tag="fin")
            nc.vector.tensor_scalar_add(fin[:], tl[:], ofs[:, :1])
            nc.sync.dma_start(out=out[gb * 128:(gb + 1) * 128, :], in_=fin[:])

    nc.compile()
    return nc


_CACHE = {}


def kernel(edge_index, batch, W0, b0, W1, b1, W2, b2, Wp, bp):
    edge_index = np.asarray(edge_index, dtype=np.int32)
    batch = np.asarray(batch, dtype=np.int32)
    W0 = np.asarray(W0, dtype=np.float32)
    b0 = np.asarray(b0, dtype=np.float32)
    W1 = np.asarray(W1, dtype=np.float32)
    b1 = np.asarray(b1, dtype=np.float32)
    W2 = np.asarray(W2, dtype=np.float32)
    b2 = np.asarray(b2, dtype=np.float32)
    Wp = np.asarray(Wp, dtype=np.float32)
    bp = np.asarray(bp, dtype=np.float32)

    key = hash((edge_index.tobytes(), batch.tobytes()))
    if key not in _CACHE:
        sched, percore, invcnt = preprocess(edge_index, batch)
        nc = build_program(sched)
        _CACHE[key] = (sched, percore, invcnt, nc)
    sched, percore, invcnt, nc = _CACHE[key]

    consts = {
        "W1": W1.astype(NP_BF16),
        "W2": W2.astype(NP_BF16),
        "Wp": Wp,
        "W0r": np.tile(W0.reshape(1, H), (128, 1)),
        "b0r": np.tile(b0.reshape(1, H), (128, 1)),
        "b1r": np.tile(b1.reshape(1, H), (128, 1)),
        "b2r": np.tile(b2.reshape(1, H), (128, 1)),
        "bpr": np.tile(bp.reshape(1, C), (128, 1)),
        "ident": np.eye(128, dtype=np.float32),
        "identb": np.eye(128, dtype=np.float32).astype(NP_BF16),
        "iotar": np.tile(
            np.arange(128, dtype=np.float32).astype(NP_BF16).reshape(1, 128),
            (128, 1),
        ),
        "giota": np.tile(np.arange(G, dtype=np.float32).reshape(1, G), (128, 1)),
        "invc": invcnt,
    }
    consts = {k: np.ascontiguousarray(v) for k, v in consts.items()}

    in_maps = []
    for c in range(NC):
        m = {
            "gidx": percore["gidx"][c],
            "dloc": percore["dloc"][c],
            "dinv": percore["dinv_pj"][c],
            "acol": percore["a_pj"][c],
            "selfw": percore["selfw_pj"][c],
            "batchf": percore["batc_pj"][c],
        }
        m.update(consts)
        in_maps.append(m)

    import os
    trace = bool(int(os.environ.get("KGCN_TRACE", "0")))
    res = run_bass_kernel_spmd(
        nc, in_maps, core_ids=list(range(NC)), trace=trace
    )
    kernel.last_results = res
    return res.results[0]["out"]
